# revision 29
# baseline (speedup 1.0000x reference)
"""Trainium2 Bass kernel for nn_FFTResonanceLookup.

Full inputs: selections (8,16,4,512) f32, items (512,771) f32.
Output: (8,16,4,32768) f32, unit-normalized along samples.

Data-parallel over the 512 (b,e,x) rows -> 64 rows/core x 8 cores.
Layout: per row 129 spec columns [z, t=1..128] (z = zero column for the
overlap-add W2 path at frame 0), so FREE = 64*129 = 8256 per bin-half.
Spectra: amp args + phase args via selection matmuls (f32r), exp/sin on
the scalar engine; cos comes from the half-angle identity
cos(2*pi*u) = 1 - 2*sin(pi*u)^2 (keeps the Sin table in-domain).
Stage-4 irfft+hann+OLA matmuls are interleaved into the stage-3 piece
loop so the PE stays dense (HAM warm). Normalization on host.
"""

import math
from contextlib import ExitStack

import numpy as np

N_ITEMS = 512
N_COEFFS = 771
CBINS = 257
WIN = 512
STEP = 256
NFR = 128
NT = 129             # per-row spec cols: [z, t=1..128]
R_PER_CORE = 64
N_CORES = 8
NROWS = 512
FREE = R_PER_CORE * NT  # 8256
PIECE = 512
# piece k covers asl cols [512k, 512k+wk); 16 full + one 64-wide tail
PIECES = [(k * PIECE, min(PIECE, FREE - k * PIECE)) for k in range((FREE + PIECE - 1) // PIECE)]

_BUILT = None


def _build_consts():
    import ml_dtypes
    hann = np.hanning(WIN)
    k = np.arange(CBINS)[:, None]
    n = np.arange(WIN)[None, :]
    ang = 2.0 * np.pi * k * n / WIN
    Cm = np.cos(ang) / WIN * np.where((k >= 1) & (k <= 255), 2.0, 1.0)
    Sm = -np.sin(ang) / WIN * np.where((k >= 1) & (k <= 255), 2.0, 0.0)
    Cw = Cm * hann[None, :]
    Sw = Sm * hann[None, :]
    W1 = np.concatenate([Cw[:256, :STEP], Sw[:256, :STEP]], 0)   # (512,256)
    W2 = np.concatenate([Cw[:256, STEP:], Sw[:256, STEP:]], 0)   # (512,256)
    wm = np.zeros((128, 8 * 256))
    for j in range(4):
        wm[:, 256 * j:256 * (j + 1)] = W1[128 * j:128 * (j + 1), :]
        wm[:, 256 * (j + 4):256 * (j + 5)] = W2[128 * j:128 * (j + 1), :]
    # RE tiles hold -AMP*cos (from (2sh^2-1)*AMP); negate the Cw chunks
    wm[:, 0:512] *= -1.0
    wm[:, 1024:1536] *= -1.0
    # bin-256 (re) rides the dead im-bin-0 slot: chunk 2 = imA@W1, chunk 6 = imA@W2
    # (IM path is not sign-flipped, so these stay positive)
    wm[0, 256 * 2:256 * 3] = Cw[256, :STEP]
    wm[0, 256 * 6:256 * 7] = Cw[256, STEP:]
    # selection matrix: rows 0-63 t-values, rows 64-127 ones; col r*129+k (k>=1)
    sel_t = np.zeros((64, FREE))
    sel_1 = np.zeros((64, FREE))
    for r in range(64):
        sel_t[r, r * NT + 1:r * NT + NT] = np.arange(1, NT, dtype=np.float64)
        sel_1[r, r * NT + 1:r * NT + NT] = 1.0
    ampsel = np.concatenate([sel_t, sel_1], 0)                   # (128,8256)
    return (wm.astype(ml_dtypes.bfloat16), ampsel.astype(np.float32))


def _kernel_body(ctx: ExitStack, tc, out_d, sel_d, items_d, wm_d,
                 ampsel_d, ident_d, negi_d):
    import concourse.mybir as mybir

    nc = tc.nc
    f32 = mybir.dt.float32
    f32r = mybir.dt.float32r
    bf16 = mybir.dt.bfloat16
    fp16 = mybir.dt.float16
    AF = mybir.ActivationFunctionType
    OP = mybir.AluOpType
    PI = math.pi
    MAGIC = 12582912.0

    const = ctx.enter_context(tc.tile_pool(name="const", bufs=1))
    persist = ctx.enter_context(tc.tile_pool(name="persist", bufs=1))
    scratch = ctx.enter_context(tc.tile_pool(name="scratch", bufs=3))

    # ---- constants + resident asl ----
    ident = const.tile([128, 128], f32)
    nc.sync.dma_start(ident[:], ident_d[:])
    wm = const.tile([128, 2048], bf16)
    nc.sync.dma_start(wm[:], wm_d[:])
    negi = const.tile([128, 128], fp16)
    nc.sync.dma_start(negi[:], negi_d[:])
    asl = persist.tile([128, FREE], f32r, name="asl")
    bias_half = const.tile([128, 1], f32)
    nc.vector.memset(bias_half[:], 0.5)

    # ---- setup: coeffs + per-bin transforms (scoped pools) ----
    ipc = tc.tile_pool(name="itemsp", bufs=1)
    ip = ipc.__enter__()
    pc = tc.tile_pool(name="pcoef", bufs=2, space="PSUM")
    pcp = pc.__enter__()
    _pn = [0]

    def ctile(shape):
        _pn[0] += 1
        return pcp.tile(shape, f32, tag="pc", name=f"pc{_pn[0]}")

    sel_t = ip.tile([64, 512], f32)
    nc.sync.dma_start(sel_t[:], sel_d[:])
    rs0 = ip.tile([64, 512], f32)
    nc.vector.tensor_relu(rs0[:], sel_t[:])

    items_v = items_d.rearrange("(a p) c -> a p c", p=128)
    it = []
    for kc in range(4):
        itk = ip.tile([128, N_COEFFS], f32, name=f"it{kc}")
        nc.sync.dma_start(itk[:], items_v[kc])
        it.append(itk)

    rsT = []
    for kc in range(4):
        pt = ctile([128, 64])
        nc.tensor.transpose(pt[:], rs0[:, kc * 128:(kc + 1) * 128],
                            ident[0:64, 0:64])
        st = persist.tile([128, 64], f32, name=f"rsT{kc}")
        nc.vector.tensor_copy(st[:], pt[:])
        rsT.append(st)

    coefA = persist.tile([64, N_COEFFS], f32)
    pA1 = ctile([64, 512])
    for kc in range(4):
        nc.tensor.matmul(pA1[:], rsT[kc][:], it[kc][:, 0:512],
                         start=(kc == 0), stop=(kc == 3))
    pA2 = ctile([64, 259])
    for kc in range(4):
        nc.tensor.matmul(pA2[:], rsT[kc][:], it[kc][:, 512:771],
                         start=(kc == 0), stop=(kc == 3))
    nc.vector.tensor_copy(coefA[:, 0:512], pA1[:])
    nc.vector.tensor_copy(coefA[:, 512:771], pA2[:])

    secT = {}
    for name, base in (("mag", 0), ("st", 2 * CBINS), ("ph", CBINS)):
        for c in range(2):
            pB = ctile([128, 64])
            lo = base + 128 * c
            for kc in range(4):
                nc.tensor.matmul(pB[:], it[kc][:, lo:lo + 128], rsT[kc][:],
                                 start=(kc == 0), stop=(kc == 3))
            sb = persist.tile([128, 64], f32, name=f"secT_{name}{c}")
            nc.vector.tensor_copy(sb[:], pB[:])
            secT[(name, c)] = sb

    # sigmoid/tanh via exp + reciprocal so ALL setup ACTs live in the
    # natural_log_exp table set (no table ping-pong with the scheduler)
    _sn = [0]

    def _sig(dst, src, p, n, scale):
        # dst = 1 / (1 + exp(scale * src))
        _sn[0] += 1
        e = scratch.tile([p, n], f32, tag=f"sg{p}x{n}", name=f"sg{_sn[0]}")
        nc.scalar.activation(e[:], src, AF.Exp, scale=scale)
        f = scratch.tile([p, n], f32, tag=f"sf{p}x{n}", name=f"sf{_sn[0]}")
        nc.vector.tensor_scalar(f[:], e[:], 1.0, None, OP.add)
        nc.vector.reciprocal(dst, f[:])

    sig = {}
    for c in range(2):
        sig[("mag", c)] = persist.tile([128, 64], f32, name=f"sig_mag{c}")
        _sig(sig[("mag", c)][:], secT[("mag", c)][:], 128, 64, -1.0)
        sig[("st", c)] = persist.tile([128, 64], f32, name=f"sig_st{c}")
        _sig(sig[("st", c)][:], secT[("st", c)][:], 128, 64, -1.0)
    b256 = persist.tile([64, 3], f32)
    _sig(b256[:, 0:1], coefA[:, 256:257], 64, 1, -1.0)
    _sig(b256[:, 2:3], coefA[:, 770:771], 64, 1, -1.0)

    # thh2 = tanh(ph)/2 = 0.5 - 1/(1+exp(2*ph))
    thh2 = {}
    for c in range(2):
        g = scratch.tile([128, 64], f32, tag="sr", name=f"sr{c}")
        _sig(g[:], secT[("ph", c)][:], 128, 64, 2.0)
        t2 = persist.tile([128, 64], f32, name=f"thh2{c}")
        nc.vector.tensor_scalar(t2[:], g[:], -1.0, 0.5, OP.mult, OP.add)
        thh2[c] = t2
    g9 = scratch.tile([64, 1], f32, tag="sr9t", name="sr9")
    _sig(g9[:], coefA[:, 513:514], 64, 1, 2.0)
    ph256 = persist.tile([64, 1], f32)
    nc.vector.tensor_scalar(ph256[:], g9[:], -1.0, 0.5, OP.mult, OP.add)

    tvi = persist.tile([64, NT], mybir.dt.int32)
    nc.gpsimd.iota(tvi[:], pattern=[[1, NT]], base=0, channel_multiplier=0)
    tvec = persist.tile([64, NT], f32)
    nc.vector.tensor_copy(tvec[:], tvi[:])

    # Ln batch (same natural_log_exp set)
    logm, lnst = {}, {}
    for c in range(2):
        lm = persist.tile([128, 64], f32, name=f"logm{c}")
        nc.scalar.activation(lm[:], sig[("mag", c)][:], AF.Ln,
                             bias=bias_half[:], scale=0.49995)
        logm[c] = lm
        ls = persist.tile([128, 64], f32, name=f"lnst{c}")
        nc.scalar.activation(ls[:], sig[("st", c)][:], AF.Ln)
        lnst[c] = ls
    logm256 = persist.tile([64, 1], f32)
    nc.scalar.activation(logm256[:], b256[:, 0:1], AF.Ln,
                         bias=bias_half[0:64], scale=0.49995)
    lnst256 = persist.tile([64, 1], f32)
    nc.scalar.activation(lnst256[:], b256[:, 2:3], AF.Ln)

    # transposes into arg-matmul weight layout
    ampx, thh2T = {}, {}
    for c in range(2):
        ax = persist.tile([128, 128], f32r, name=f"ampx{c}")
        ptr = ctile([64, 128])
        nc.tensor.transpose(ptr[:], logm[c][:], ident[:])
        nc.vector.tensor_copy(ax[0:64, :], ptr[:])
        ptr2 = ctile([64, 128])
        nc.tensor.transpose(ptr2[:], lnst[c][:], ident[:])
        nc.vector.tensor_copy(ax[64:128, :], ptr2[:])
        ampx[c] = ax
        ptr3 = ctile([64, 128])
        nc.tensor.transpose(ptr3[:], thh2[c][:], ident[:])
        tt = persist.tile([64, 128], f32r, name=f"thh2T{c}")
        nc.vector.tensor_copy(tt[:], ptr3[:])
        thh2T[c] = tt

    # resident asl loads (consumed from phase A onward)
    for (c0, w) in PIECES:
        nc.gpsimd.dma_start(asl[:, c0:c0 + w], ampsel_d[:, c0:c0 + w])

    pc.__exit__(None, None, None)   # free setup PSUM banks
    ipc.__exit__(None, None, None)  # free items/sel SBUF

    # ---- big SBUF tiles ----
    # AMP interleaved: piece (k,c) at cols 1024k+512c (never read by DFT)
    AMPB = persist.tile([128, 2 * FREE], bf16, name="AMPB")
    RE = [persist.tile([128, FREE], bf16, name=f'RE{i}') for i in range(2)]
    IM = [persist.tile([128, FREE], bf16, name=f'IM{i}') for i in range(2)]

    pargs = ctx.enter_context(tc.tile_pool(name="pargs", bufs=2, space="PSUM"))
    pdft = ctx.enter_context(tc.tile_pool(name="pdft", bufs=3, space="PSUM"))
    ostage = ctx.enter_context(tc.tile_pool(name="ostage", bufs=3))

    # ---- phase A: amp args -> Exp ----
    for ki, (c0, w) in enumerate(PIECES):
        _pn[0] += 1
        pa = pargs.tile([128, 2 * PIECE], f32, tag="pa", name=f"pa{_pn[0]}")
        for c in range(2):
            nc.tensor.matmul(pa[:, c * PIECE:c * PIECE + w], ampx[c][:],
                             asl[:, c0:c0 + w], start=True, stop=True)
        if w == PIECE:
            nc.scalar.activation(AMPB[:, 2 * c0:2 * c0 + 2 * PIECE], pa[:],
                                 AF.Exp)
        else:
            for c in range(2):
                nc.scalar.activation(
                    AMPB[:, 2 * c0 + c * w:2 * c0 + (c + 1) * w],
                    pa[:, c * PIECE:c * PIECE + w], AF.Exp)
    a256 = scratch.tile([64, NT], f32, tag="a256")
    nc.vector.tensor_scalar(a256[:], tvec[:], logm256[:], lnst256[:],
                            OP.mult, OP.add)
    A256 = persist.tile([64, NT], bf16)
    nc.scalar.activation(A256[:], a256[:], AF.Exp)

    # bin-256 spec -> IM[0] partition 0 (dead im-bin-0 slot).
    # Gate the Sin behind the last Exp piece so the scheduler cannot hoist
    # it into the Exp batch (extra table loads otherwise).
    p9 = persist.tile([64, NT], f32, name="p9")
    nc.vector.tensor_scalar(p9[:], tvec[:], ph256[:], 0.25, OP.mult, OP.add)
    r9 = persist.tile([64, NT], f32, name="r9")
    nc.vector.tensor_scalar(r9[:], p9[:], MAGIC, MAGIC, OP.add, OP.subtract)
    u9 = persist.tile([64, NT], f32, name="u9")
    nc.vector.scalar_tensor_tensor(u9[:], p9[:], 1.0, r9[:],
                                   OP.mult, OP.subtract)
    gate9 = persist.tile([64, 1], f32, name="gate9")
    nc.vector.tensor_scalar(gate9[:], AMPB[0:64, 2 * FREE - 1:2 * FREE],
                            0.0, None, OP.mult)
    u9g = persist.tile([64, NT], f32, name="u9g")
    nc.vector.tensor_scalar(u9g[:], u9[:], gate9[:], None, OP.add)
    cs9 = persist.tile([64, NT], bf16, name="cs9")
    nc.scalar.activation(cs9[:], u9g[:], AF.Sin, scale=2.0 * PI)
    S256 = persist.tile([64, NT], bf16)
    nc.vector.tensor_tensor(S256[:], A256[:], cs9[:], OP.mult)
    nc.vector.memset(S256[:, 0:1], 0.0)
    im0v = IM[0][0:1, :].rearrange("o (r t) -> o r t", t=NT)

    REv = [RE[c][:, :].rearrange("p (r t) -> p r t", t=NT) for c in range(2)]
    specs = [RE[0], RE[1], IM[0], IM[1]]
    _cp = [0]

    def dft_group(r0, nr):
        _pn[0] += 1
        po = pdft.tile([128, nr * 256], f32, tag="po", name=f"po{_pn[0]}")
        for i in range(nr):
            r = r0 + i
            ow = slice(i * 256, (i + 1) * 256)
            for j in range(4):
                nc.tensor.matmul(po[:, ow],
                                 specs[j][:, r * NT + 1:r * NT + NT],
                                 wm[:, j * 256:(j + 1) * 256],
                                 start=(j == 0), stop=False)
            for j in range(4):
                nc.tensor.matmul(po[:, ow],
                                 specs[j][:, r * NT:r * NT + NFR],
                                 wm[:, (j + 4) * 256:(j + 5) * 256],
                                 start=False, stop=(j == 3))
        ot = ostage.tile([128, nr * 256], f32, tag="ot", name=f"ot{_pn[0]}")
        _cp[0] += 1
        if _cp[0] % 2 == 0:
            nc.vector.tensor_copy(ot[:], po[:])
        else:
            nc.scalar.copy(ot[:], po[:])
        dst = out_d[r0:r0 + nr, :].rearrange("r (p s) -> p r s", s=256)
        src = ot[:, 0:nr * 256].rearrange("p (r s) -> p r s", s=256)
        nc.sync.dma_start(dst, src)

    # ---- phase B: sin args -> sin/cos -> RE/IM, DFT interleaved ----
    r_done = -1
    prev = []   # deferred per-piece ops from previous round
    for ki, (c0, w) in enumerate(PIECES):
        _pn[0] += 1
        pt = pargs.tile([128, 2 * PIECE], f32, tag="pa", name=f"pt{_pn[0]}")
        for c in range(2):
            nc.tensor.matmul(pt[:, c * PIECE:c * PIECE + w],
                             thh2T[c][:], asl[0:64, c0:c0 + w],
                             start=True, stop=True)
        # DFT lags two rounds so its RE/IM inputs are ready when the PE
        # reaches these matmuls (depth-2 pipeline; avoids PE stalls)
        r_hi = (c0 - PIECE) // NT - 1 if ki > 1 else -1
        while r_done + 2 <= r_hi:
            dft_group(r_done + 1, 2)
            r_done += 2
        # range reduction: u = theta - round(theta), in-place in PSUM
        rr = scratch.tile([128, 2 * PIECE], fp16, tag="rr")
        for c in range(2):
            cw = slice(c * PIECE, c * PIECE + w)
            nc.vector.tensor_scalar(rr[:, cw], pt[:, cw], MAGIC, MAGIC,
                                    OP.add, OP.subtract)
        for c in range(2):
            cw = slice(c * PIECE, c * PIECE + w)
            nc.tensor.matmul(pt[:, cw], negi[:], rr[:, cw], start=False,
                             stop=True, skip_group_check=True)
        s2 = scratch.tile([128, 2 * PIECE], bf16, tag="s2")
        sh = scratch.tile([128, 2 * PIECE], bf16, tag="sh")
        if w == PIECE:
            nc.scalar.activation(s2[:], pt[:], AF.Sin, scale=2.0 * PI)
            nc.scalar.activation(sh[:], pt[:], AF.Sin, scale=PI)
        else:
            for c in range(2):
                cw = slice(c * PIECE, c * PIECE + w)
                nc.scalar.activation(s2[:, cw], pt[:, cw], AF.Sin,
                                     scale=2.0 * PI)
                nc.scalar.activation(sh[:, cw], pt[:, cw], AF.Sin, scale=PI)
        # cos(2pi u) = 1 - 2 sh^2; RE' = (2sh^2 - 1)*AMP = -AMP*cos
        # (sign absorbed into negated Cw chunks of wm); X = 2 sh^2 via Square
        X = scratch.tile([128, 2 * PIECE], bf16, tag="X")
        if w == PIECE:
            nc.scalar.activation(X[:], sh[:], AF.Square,
                                 scale=math.sqrt(2.0))
        else:
            for c in range(2):
                cw = slice(c * PIECE, c * PIECE + w)
                nc.scalar.activation(X[:, cw], sh[:, cw], AF.Square,
                                     scale=math.sqrt(2.0))
        for c in range(2):
            cw = slice(c * PIECE, c * PIECE + w)
            aw = slice(2 * c0 + c * w, 2 * c0 + (c + 1) * w)
            nc.vector.scalar_tensor_tensor(RE[c][:, c0:c0 + w], X[:, cw],
                                           -1.0, AMPB[:, aw],
                                           OP.add, OP.mult)
            nc.gpsimd.tensor_tensor(IM[c][:, c0:c0 + w],
                                    AMPB[:, aw], s2[:, cw], OP.mult)
            if c == 0:
                # restore bin-256 spec on the (clobbered) im-bin-0 slot
                r_a = c0 // NT
                r_b = (c0 + w - 1) // NT
                nc.gpsimd.dma_start(im0v[:, r_a:r_b + 1, 0:NT],
                                    S256[r_a:r_b + 1, :])
            # zero the z-columns (t=0) of RE covered by this piece
            rz0 = (c0 + NT - 1) // NT
            rz1 = (c0 + w - 1) // NT
            if rz1 >= rz0:
                nc.gpsimd.memset(REv[c][:, rz0:rz1 + 1, 0:1], 0.0)
    # remaining rows
    while r_done + 1 < R_PER_CORE:
        nr = min(2, R_PER_CORE - 1 - r_done)
        dft_group(r_done + 1, nr)
        r_done += nr


def _build():
    global _BUILT
    if _BUILT is not None:
        return _BUILT
    import concourse.bacc as bacc
    import concourse.mybir as mybir
    import concourse.tile as tile

    wm_np, ampsel_np = _build_consts()
    ident_np = np.eye(128, dtype=np.float32)
    negi_np = -np.eye(128, dtype=np.float16)

    nc = bacc.Bacc("TRN2", target_bir_lowering=False, debug=False,
                   num_devices=N_CORES)
    f32 = mybir.dt.float32
    bf16 = mybir.dt.bfloat16
    sel_d = nc.dram_tensor("sel", [R_PER_CORE, N_ITEMS], f32,
                           kind="ExternalInput").ap()
    items_d = nc.dram_tensor("items", [N_ITEMS, N_COEFFS], f32,
                             kind="ExternalInput").ap()
    wm_d = nc.dram_tensor("wm", list(wm_np.shape), bf16,
                          kind="ExternalInput").ap()
    ampsel_d = nc.dram_tensor("ampsel", list(ampsel_np.shape),
                              mybir.dt.float32r, kind="ExternalInput").ap()
    ident_d = nc.dram_tensor("ident", [128, 128], f32,
                             kind="ExternalInput").ap()
    negi_d = nc.dram_tensor("negi", [128, 128], mybir.dt.float16,
                            kind="ExternalInput").ap()
    out_d = nc.dram_tensor("out", [R_PER_CORE, NFR * STEP], f32,
                           kind="ExternalOutput").ap()

    with tile.TileContext(nc) as tc:
        with ExitStack() as ctx:
            _kernel_body(ctx, tc, out_d, sel_d, items_d, wm_d,
                         ampsel_d, ident_d, negi_d)
    nc.compile()

    _BUILT = (nc, wm_np, ampsel_np, ident_np, negi_np)
    return _BUILT


def kernel(selections: np.ndarray, items: np.ndarray) -> np.ndarray:
    from concourse.bass_utils import run_bass_kernel_spmd

    nc, wm_np, ampsel_np, ident_np, negi_np = _build()
    sel_flat = np.ascontiguousarray(
        np.asarray(selections).reshape(NROWS, N_ITEMS).astype(np.float32))
    items_f = np.ascontiguousarray(np.asarray(items).astype(np.float32))
    in_maps = []
    for c in range(N_CORES):
        in_maps.append({
            "sel": sel_flat[c * R_PER_CORE:(c + 1) * R_PER_CORE],
            "items": items_f,
            "wm": wm_np,
            "ampsel": ampsel_np,
            "ident": ident_np,
            "negi": negi_np,
        })
    res = run_bass_kernel_spmd(nc, in_maps, core_ids=list(range(N_CORES)))
    rows = np.concatenate([res.results[c]["out"] for c in range(N_CORES)], 0)
    norms = np.linalg.norm(rows.astype(np.float64), axis=-1, keepdims=True)
    rows = rows / (norms + 1e-8)
    sh = np.asarray(selections).shape
    return rows.reshape(sh[0], sh[1], sh[2], NFR * STEP).astype(np.float32)


# revision 32
# speedup vs baseline: 1.0685x; 1.0685x over previous
"""Trainium2 Bass kernel for nn_FFTResonanceLookup.

Full inputs: selections (8,16,4,512) f32, items (512,771) f32.
Output: (8,16,4,32768) f32, unit-normalized along samples.

Data-parallel over the 512 (b,e,x) rows -> 64 rows/core x 8 cores.
Layout: per row 129 spec columns [z, t=1..128] (z = zero column for the
overlap-add W2 path at frame 0), so FREE = 64*129 = 8256 per bin-half.
Spectra: amp args + phase args via selection matmuls (f32r), exp/sin on
the scalar engine; cos comes from the half-angle identity
cos(2*pi*u) = 1 - 2*sin(pi*u)^2 (keeps the Sin table in-domain).
Stage-4 irfft+hann+OLA matmuls are interleaved into the stage-3 piece
loop so the PE stays dense (HAM warm). Normalization on host.
"""

import math
from contextlib import ExitStack

import numpy as np

N_ITEMS = 512
N_COEFFS = 771
CBINS = 257
WIN = 512
STEP = 256
NFR = 128
NT = 129             # per-row spec cols: [z, t=1..128]
R_PER_CORE = 64
N_CORES = 8
NROWS = 512
FREE = R_PER_CORE * NT  # 8256
PIECE = 512
# piece k covers asl cols [512k, 512k+wk); 16 full + one 64-wide tail
PIECES = [(k * PIECE, min(PIECE, FREE - k * PIECE)) for k in range((FREE + PIECE - 1) // PIECE)]

_BUILT = None


def _build_consts():
    import ml_dtypes
    hann = np.hanning(WIN)
    k = np.arange(CBINS)[:, None]
    n = np.arange(WIN)[None, :]
    ang = 2.0 * np.pi * k * n / WIN
    Cm = np.cos(ang) / WIN * np.where((k >= 1) & (k <= 255), 2.0, 1.0)
    Sm = -np.sin(ang) / WIN * np.where((k >= 1) & (k <= 255), 2.0, 0.0)
    Cw = Cm * hann[None, :]
    Sw = Sm * hann[None, :]
    W1 = np.concatenate([Cw[:256, :STEP], Sw[:256, :STEP]], 0)   # (512,256)
    W2 = np.concatenate([Cw[:256, STEP:], Sw[:256, STEP:]], 0)   # (512,256)
    wm = np.zeros((128, 8 * 256))
    for j in range(4):
        wm[:, 256 * j:256 * (j + 1)] = W1[128 * j:128 * (j + 1), :]
        wm[:, 256 * (j + 4):256 * (j + 5)] = W2[128 * j:128 * (j + 1), :]
    # RE tiles hold -AMP*cos (from (2sh^2-1)*AMP); negate the Cw chunks
    wm[:, 0:512] *= -1.0
    wm[:, 1024:1536] *= -1.0
    # bin-256 (re) rides the dead im-bin-0 slot: chunk 2 = imA@W1, chunk 6 = imA@W2
    # (IM path is not sign-flipped, so these stay positive)
    wm[0, 256 * 2:256 * 3] = Cw[256, :STEP]
    wm[0, 256 * 6:256 * 7] = Cw[256, STEP:]
    # selection matrix: rows 0-63 t-values, rows 64-127 ones; col r*129+k (k>=1)
    sel_t = np.zeros((64, FREE))
    sel_1 = np.zeros((64, FREE))
    for r in range(64):
        sel_t[r, r * NT + 1:r * NT + NT] = np.arange(1, NT, dtype=np.float64)
        sel_1[r, r * NT + 1:r * NT + NT] = 1.0
    ampsel = np.concatenate([sel_t, sel_1], 0)                   # (128,8256)
    return (wm.astype(ml_dtypes.bfloat16), ampsel.astype(np.float32))


def _kernel_body(ctx: ExitStack, tc, out_d, sel_d, items_d, wm_d,
                 ampsel_d, ident_d, negi_d):
    import concourse.mybir as mybir

    nc = tc.nc
    f32 = mybir.dt.float32
    f32r = mybir.dt.float32r
    bf16 = mybir.dt.bfloat16
    fp16 = mybir.dt.float16
    AF = mybir.ActivationFunctionType
    OP = mybir.AluOpType
    PI = math.pi
    MAGIC = 12582912.0

    const = ctx.enter_context(tc.tile_pool(name="const", bufs=1))
    persist = ctx.enter_context(tc.tile_pool(name="persist", bufs=1))
    scratch = ctx.enter_context(tc.tile_pool(name="scratch", bufs=3))

    # ---- constants + resident asl ----
    ident = const.tile([128, 128], f32)
    nc.sync.dma_start(ident[:], ident_d[:])
    wm = const.tile([128, 2048], bf16)
    nc.sync.dma_start(wm[:], wm_d[:])
    negi = const.tile([128, 128], fp16)
    nc.sync.dma_start(negi[:], negi_d[:])
    asl = persist.tile([128, FREE], f32r, name="asl")
    bias_half = const.tile([128, 1], f32)
    nc.vector.memset(bias_half[:], 0.5)

    # ---- setup: coeffs + per-bin transforms (scoped pools) ----
    ipc = tc.tile_pool(name="itemsp", bufs=1)
    ip = ipc.__enter__()
    pc = tc.tile_pool(name="pcoef", bufs=2, space="PSUM")
    pcp = pc.__enter__()
    _pn = [0]

    def ctile(shape):
        _pn[0] += 1
        return pcp.tile(shape, f32, tag="pc", name=f"pc{_pn[0]}")

    sel_t = ip.tile([64, 512], f32)
    nc.sync.dma_start(sel_t[:], sel_d[:])
    rs0 = ip.tile([64, 512], f32)
    nc.vector.tensor_relu(rs0[:], sel_t[:])

    items_v = items_d.rearrange("(a p) c -> a p c", p=128)
    it = []
    for kc in range(4):
        itk = ip.tile([128, N_COEFFS], f32, name=f"it{kc}")
        nc.sync.dma_start(itk[:], items_v[kc])
        it.append(itk)

    rsT = []
    for kc in range(4):
        pt = ctile([128, 64])
        nc.tensor.transpose(pt[:], rs0[:, kc * 128:(kc + 1) * 128],
                            ident[0:64, 0:64])
        st = persist.tile([128, 64], f32, name=f"rsT{kc}")
        nc.vector.tensor_copy(st[:], pt[:])
        rsT.append(st)

    coefA = persist.tile([64, N_COEFFS], f32)
    pA1 = ctile([64, 512])
    for kc in range(4):
        nc.tensor.matmul(pA1[:], rsT[kc][:], it[kc][:, 0:512],
                         start=(kc == 0), stop=(kc == 3))
    pA2 = ctile([64, 259])
    for kc in range(4):
        nc.tensor.matmul(pA2[:], rsT[kc][:], it[kc][:, 512:771],
                         start=(kc == 0), stop=(kc == 3))
    nc.vector.tensor_copy(coefA[:, 0:512], pA1[:])
    nc.vector.tensor_copy(coefA[:, 512:771], pA2[:])

    secT = {}
    for name, base in (("mag", 0), ("st", 2 * CBINS), ("ph", CBINS)):
        for c in range(2):
            pB = ctile([128, 64])
            lo = base + 128 * c
            for kc in range(4):
                nc.tensor.matmul(pB[:], it[kc][:, lo:lo + 128], rsT[kc][:],
                                 start=(kc == 0), stop=(kc == 3))
            sb = persist.tile([128, 64], f32, name=f"secT_{name}{c}")
            nc.vector.tensor_copy(sb[:], pB[:])
            secT[(name, c)] = sb

    # sigmoid/tanh via exp + reciprocal so ALL setup ACTs live in the
    # natural_log_exp table set (no table ping-pong with the scheduler)
    _sn = [0]

    def _sig(dst, src, p, n, scale):
        # dst = 1 / (1 + exp(scale * src))
        _sn[0] += 1
        e = scratch.tile([p, n], f32, tag=f"sg{p}x{n}", name=f"sg{_sn[0]}")
        nc.scalar.activation(e[:], src, AF.Exp, scale=scale)
        f = scratch.tile([p, n], f32, tag=f"sf{p}x{n}", name=f"sf{_sn[0]}")
        nc.vector.tensor_scalar(f[:], e[:], 1.0, None, OP.add)
        nc.vector.reciprocal(dst, f[:])

    sig = {}
    for c in range(2):
        sig[("mag", c)] = persist.tile([128, 64], f32, name=f"sig_mag{c}")
        _sig(sig[("mag", c)][:], secT[("mag", c)][:], 128, 64, -1.0)
        sig[("st", c)] = persist.tile([128, 64], f32, name=f"sig_st{c}")
        _sig(sig[("st", c)][:], secT[("st", c)][:], 128, 64, -1.0)
    b256 = persist.tile([64, 3], f32)
    _sig(b256[:, 0:1], coefA[:, 256:257], 64, 1, -1.0)
    _sig(b256[:, 2:3], coefA[:, 770:771], 64, 1, -1.0)

    # thh2 = tanh(ph)/2 = 0.5 - 1/(1+exp(2*ph))
    thh2 = {}
    for c in range(2):
        g = scratch.tile([128, 64], f32, tag="sr", name=f"sr{c}")
        _sig(g[:], secT[("ph", c)][:], 128, 64, 2.0)
        t2 = persist.tile([128, 64], f32, name=f"thh2{c}")
        nc.vector.tensor_scalar(t2[:], g[:], -1.0, 0.5, OP.mult, OP.add)
        thh2[c] = t2
    g9 = scratch.tile([64, 1], f32, tag="sr9t", name="sr9")
    _sig(g9[:], coefA[:, 513:514], 64, 1, 2.0)
    ph256 = persist.tile([64, 1], f32)
    nc.vector.tensor_scalar(ph256[:], g9[:], -1.0, 0.5, OP.mult, OP.add)

    tvi = persist.tile([64, NT], mybir.dt.int32)
    nc.gpsimd.iota(tvi[:], pattern=[[1, NT]], base=0, channel_multiplier=0)
    tvec = persist.tile([64, NT], f32)
    nc.vector.tensor_copy(tvec[:], tvi[:])

    # Ln batch. The compiler puts Ln and Exp in different table sets, so
    # gate every Ln's bias on the LAST exp-chain output (thh2[1]/ph256):
    # the scheduler then cannot interleave Lns into the exp batch.
    biasg = persist.tile([128, 1], f32, name="biasg")
    nc.vector.tensor_scalar(biasg[:], thh2[1][:, 0:1], 0.0, 0.5,
                            OP.mult, OP.add)
    biasg0 = persist.tile([128, 1], f32, name="biasg0")
    nc.vector.tensor_scalar(biasg0[:], thh2[1][:, 0:1], 0.0, None, OP.mult)
    b64g = persist.tile([64, 1], f32, name="b64g")
    nc.vector.tensor_scalar(b64g[:], ph256[:], 0.0, 0.5, OP.mult, OP.add)
    b64g0 = persist.tile([64, 1], f32, name="b64g0")
    nc.vector.tensor_scalar(b64g0[:], ph256[:], 0.0, None, OP.mult)
    logm, lnst = {}, {}
    for c in range(2):
        lm = persist.tile([128, 64], f32, name=f"logm{c}")
        nc.scalar.activation(lm[:], sig[("mag", c)][:], AF.Ln,
                             bias=biasg[:], scale=0.49995)
        logm[c] = lm
        ls = persist.tile([128, 64], f32, name=f"lnst{c}")
        nc.scalar.activation(ls[:], sig[("st", c)][:], AF.Ln, bias=biasg0[:])
        lnst[c] = ls
    logm256 = persist.tile([64, 1], f32)
    nc.scalar.activation(logm256[:], b256[:, 0:1], AF.Ln,
                         bias=b64g[:], scale=0.49995)
    lnst256 = persist.tile([64, 1], f32)
    nc.scalar.activation(lnst256[:], b256[:, 2:3], AF.Ln, bias=b64g0[:])

    # transposes into arg-matmul weight layout
    ampx, thh2T = {}, {}
    for c in range(2):
        ax = persist.tile([128, 128], f32r, name=f"ampx{c}")
        ptr = ctile([64, 128])
        nc.tensor.transpose(ptr[:], logm[c][:], ident[:])
        nc.vector.tensor_copy(ax[0:64, :], ptr[:])
        ptr2 = ctile([64, 128])
        nc.tensor.transpose(ptr2[:], lnst[c][:], ident[:])
        nc.vector.tensor_copy(ax[64:128, :], ptr2[:])
        ampx[c] = ax
        ptr3 = ctile([64, 128])
        nc.tensor.transpose(ptr3[:], thh2[c][:], ident[:])
        tt = persist.tile([64, 128], f32r, name=f"thh2T{c}")
        nc.vector.tensor_copy(tt[:], ptr3[:])
        thh2T[c] = tt

    # resident asl loads (consumed from phase A onward)
    for (c0, w) in PIECES:
        nc.gpsimd.dma_start(asl[:, c0:c0 + w], ampsel_d[:, c0:c0 + w])

    pc.__exit__(None, None, None)   # free setup PSUM banks
    ipc.__exit__(None, None, None)  # free items/sel SBUF

    # ---- big SBUF tiles ----
    # AMP interleaved: piece (k,c) at cols 1024k+512c (never read by DFT)
    AMPB = persist.tile([128, 2 * FREE], bf16, name="AMPB")
    RE = [persist.tile([128, FREE], bf16, name=f'RE{i}') for i in range(2)]
    IM = [persist.tile([128, FREE], bf16, name=f'IM{i}') for i in range(2)]

    pargs = ctx.enter_context(tc.tile_pool(name="pargs", bufs=3, space="PSUM"))
    pdft = ctx.enter_context(tc.tile_pool(name="pdft", bufs=2, space="PSUM"))
    ostage = ctx.enter_context(tc.tile_pool(name="ostage", bufs=3))

    # ---- phase A: amp args -> Exp ----
    for ki, (c0, w) in enumerate(PIECES):
        _pn[0] += 1
        pa = pargs.tile([128, 2 * PIECE], f32, tag="pa", name=f"pa{_pn[0]}")
        for c in range(2):
            nc.tensor.matmul(pa[:, c * PIECE:c * PIECE + w], ampx[c][:],
                             asl[:, c0:c0 + w], start=True, stop=True)
        if w == PIECE:
            nc.scalar.activation(AMPB[:, 2 * c0:2 * c0 + 2 * PIECE], pa[:],
                                 AF.Exp)
        else:
            for c in range(2):
                nc.scalar.activation(
                    AMPB[:, 2 * c0 + c * w:2 * c0 + (c + 1) * w],
                    pa[:, c * PIECE:c * PIECE + w], AF.Exp)
    a256 = scratch.tile([64, NT], f32, tag="a256")
    nc.vector.tensor_scalar(a256[:], tvec[:], logm256[:], lnst256[:],
                            OP.mult, OP.add)
    A256 = persist.tile([64, NT], bf16)
    nc.scalar.activation(A256[:], a256[:], AF.Exp)

    # bin-256 spec -> IM[0] partition 0 (dead im-bin-0 slot).
    # Gate the Sin behind the last Exp piece so the scheduler cannot hoist
    # it into the Exp batch (extra table loads otherwise).
    p9 = persist.tile([64, NT], f32, name="p9")
    nc.vector.tensor_scalar(p9[:], tvec[:], ph256[:], 0.25, OP.mult, OP.add)
    r9 = persist.tile([64, NT], f32, name="r9")
    nc.vector.tensor_scalar(r9[:], p9[:], MAGIC, MAGIC, OP.add, OP.subtract)
    u9 = persist.tile([64, NT], f32, name="u9")
    nc.vector.scalar_tensor_tensor(u9[:], p9[:], 1.0, r9[:],
                                   OP.mult, OP.subtract)
    gate9 = persist.tile([64, 1], f32, name="gate9")
    nc.vector.tensor_scalar(gate9[:], AMPB[0:64, 2 * FREE - 1:2 * FREE],
                            0.0, None, OP.mult)
    u9g = persist.tile([64, NT], f32, name="u9g")
    nc.vector.tensor_scalar(u9g[:], u9[:], gate9[:], None, OP.add)
    cs9 = persist.tile([64, NT], bf16, name="cs9")
    nc.scalar.activation(cs9[:], u9g[:], AF.Sin, scale=2.0 * PI)
    S256 = persist.tile([64, NT], bf16)
    nc.vector.tensor_tensor(S256[:], A256[:], cs9[:], OP.mult)
    nc.vector.memset(S256[:, 0:1], 0.0)
    im0v = IM[0][0:1, :].rearrange("o (r t) -> o r t", t=NT)

    REv = [RE[c][:, :].rearrange("p (r t) -> p r t", t=NT) for c in range(2)]
    specs = [RE[0], RE[1], IM[0], IM[1]]
    _cp = [0]

    def dft_group(r0, nr):
        _pn[0] += 1
        po = pdft.tile([128, nr * 256], f32, tag="po", name=f"po{_pn[0]}")
        for i in range(nr):
            r = r0 + i
            ow = slice(i * 256, (i + 1) * 256)
            for j in range(4):
                nc.tensor.matmul(po[:, ow],
                                 specs[j][:, r * NT + 1:r * NT + NT],
                                 wm[:, j * 256:(j + 1) * 256],
                                 start=(j == 0), stop=False)
            for j in range(4):
                nc.tensor.matmul(po[:, ow],
                                 specs[j][:, r * NT:r * NT + NFR],
                                 wm[:, (j + 4) * 256:(j + 5) * 256],
                                 start=False, stop=(j == 3))
        ot = ostage.tile([128, nr * 256], f32, tag="ot", name=f"ot{_pn[0]}")
        _cp[0] += 1
        if _cp[0] % 2 == 0:
            nc.vector.tensor_copy(ot[:], po[:])
        else:
            nc.scalar.copy(ot[:], po[:])
        dst = out_d[r0:r0 + nr, :].rearrange("r (p s) -> p r s", s=256)
        src = ot[:, 0:nr * 256].rearrange("p (r s) -> p r s", s=256)
        nc.sync.dma_start(dst, src)

    # ---- phase B: sin args -> sin/cos -> RE/IM, DFT interleaved ----
    r_done = -1
    prev = []   # deferred per-piece ops from previous round
    for ki, (c0, w) in enumerate(PIECES):
        _pn[0] += 1
        pt = pargs.tile([128, 2 * PIECE], f32, tag="pa", name=f"pt{_pn[0]}")
        for c in range(2):
            nc.tensor.matmul(pt[:, c * PIECE:c * PIECE + w],
                             thh2T[c][:], asl[0:64, c0:c0 + w],
                             start=True, stop=True)
        if ki == 2:
            # keep the PE's activity window busy across the A->B seam
            # (scalar is draining the Exp batch; a >3.4us PE idle here
            # would re-throttle HAM and run early DFT rounds at 1.2 GHz)
            for dk in range(40):
                nc.tensor.ldweights(wm[:, (dk % 8) * 256:(dk % 8) * 256 + 128])
        # DFT lags two rounds so its RE/IM inputs are ready when the PE
        # reaches these matmuls (depth-2 pipeline; avoids PE stalls)
        r_hi = (c0 - PIECE) // NT - 1 if ki > 1 else -1
        while r_done + 2 <= r_hi:
            dft_group(r_done + 1, 2)
            r_done += 2
        # range reduction: u = theta - round(theta), in-place in PSUM
        rr = scratch.tile([128, 2 * PIECE], fp16, tag="rr")
        for c in range(2):
            cw = slice(c * PIECE, c * PIECE + w)
            nc.vector.tensor_scalar(rr[:, cw], pt[:, cw], MAGIC, MAGIC,
                                    OP.add, OP.subtract)
        for c in range(2):
            cw = slice(c * PIECE, c * PIECE + w)
            nc.tensor.matmul(pt[:, cw], negi[:], rr[:, cw], start=False,
                             stop=True, skip_group_check=True)
        s2 = scratch.tile([128, 2 * PIECE], bf16, tag="s2")
        sh = scratch.tile([128, 2 * PIECE], bf16, tag="sh")
        if w == PIECE:
            nc.scalar.activation(s2[:], pt[:], AF.Sin, scale=2.0 * PI)
            nc.scalar.activation(sh[:], pt[:], AF.Sin, scale=PI)
        else:
            for c in range(2):
                cw = slice(c * PIECE, c * PIECE + w)
                nc.scalar.activation(s2[:, cw], pt[:, cw], AF.Sin,
                                     scale=2.0 * PI)
                nc.scalar.activation(sh[:, cw], pt[:, cw], AF.Sin, scale=PI)
        # cos(2pi u) = 1 - 2 sh^2; RE' = (2sh^2 - 1)*AMP = -AMP*cos
        # (sign absorbed into negated Cw chunks of wm); X = 2 sh^2 via Square
        X = scratch.tile([128, 2 * PIECE], bf16, tag="X")
        if w == PIECE:
            nc.scalar.activation(X[:], sh[:], AF.Square,
                                 scale=math.sqrt(2.0))
        else:
            for c in range(2):
                cw = slice(c * PIECE, c * PIECE + w)
                nc.scalar.activation(X[:, cw], sh[:, cw], AF.Square,
                                     scale=math.sqrt(2.0))
        for c in range(2):
            cw = slice(c * PIECE, c * PIECE + w)
            aw = slice(2 * c0 + c * w, 2 * c0 + (c + 1) * w)
            nc.vector.scalar_tensor_tensor(RE[c][:, c0:c0 + w], X[:, cw],
                                           -1.0, AMPB[:, aw],
                                           OP.add, OP.mult)
            nc.gpsimd.tensor_tensor(IM[c][:, c0:c0 + w],
                                    AMPB[:, aw], s2[:, cw], OP.mult)
            if c == 0:
                # restore bin-256 spec on the (clobbered) im-bin-0 slot
                r_a = c0 // NT
                r_b = (c0 + w - 1) // NT
                nc.gpsimd.dma_start(im0v[:, r_a:r_b + 1, 0:NT],
                                    S256[r_a:r_b + 1, :])
            # zero the z-columns (t=0) of RE covered by this piece
            rz0 = (c0 + NT - 1) // NT
            rz1 = (c0 + w - 1) // NT
            if rz1 >= rz0:
                nc.gpsimd.memset(REv[c][:, rz0:rz1 + 1, 0:1], 0.0)
    # remaining rows
    while r_done + 1 < R_PER_CORE:
        nr = min(2, R_PER_CORE - 1 - r_done)
        dft_group(r_done + 1, nr)
        r_done += nr


def _build():
    global _BUILT
    if _BUILT is not None:
        return _BUILT
    import concourse.bacc as bacc
    import concourse.mybir as mybir
    import concourse.tile as tile

    wm_np, ampsel_np = _build_consts()
    ident_np = np.eye(128, dtype=np.float32)
    negi_np = -np.eye(128, dtype=np.float16)

    nc = bacc.Bacc("TRN2", target_bir_lowering=False, debug=False,
                   num_devices=N_CORES)
    f32 = mybir.dt.float32
    bf16 = mybir.dt.bfloat16
    sel_d = nc.dram_tensor("sel", [R_PER_CORE, N_ITEMS], f32,
                           kind="ExternalInput").ap()
    items_d = nc.dram_tensor("items", [N_ITEMS, N_COEFFS], f32,
                             kind="ExternalInput").ap()
    wm_d = nc.dram_tensor("wm", list(wm_np.shape), bf16,
                          kind="ExternalInput").ap()
    ampsel_d = nc.dram_tensor("ampsel", list(ampsel_np.shape),
                              mybir.dt.float32r, kind="ExternalInput").ap()
    ident_d = nc.dram_tensor("ident", [128, 128], f32,
                             kind="ExternalInput").ap()
    negi_d = nc.dram_tensor("negi", [128, 128], mybir.dt.float16,
                            kind="ExternalInput").ap()
    out_d = nc.dram_tensor("out", [R_PER_CORE, NFR * STEP], f32,
                           kind="ExternalOutput").ap()

    with tile.TileContext(nc) as tc:
        with ExitStack() as ctx:
            _kernel_body(ctx, tc, out_d, sel_d, items_d, wm_d,
                         ampsel_d, ident_d, negi_d)
    nc.compile()

    _BUILT = (nc, wm_np, ampsel_np, ident_np, negi_np)
    return _BUILT


def kernel(selections: np.ndarray, items: np.ndarray) -> np.ndarray:
    from concourse.bass_utils import run_bass_kernel_spmd

    nc, wm_np, ampsel_np, ident_np, negi_np = _build()
    sel_flat = np.ascontiguousarray(
        np.asarray(selections).reshape(NROWS, N_ITEMS).astype(np.float32))
    items_f = np.ascontiguousarray(np.asarray(items).astype(np.float32))
    in_maps = []
    for c in range(N_CORES):
        in_maps.append({
            "sel": sel_flat[c * R_PER_CORE:(c + 1) * R_PER_CORE],
            "items": items_f,
            "wm": wm_np,
            "ampsel": ampsel_np,
            "ident": ident_np,
            "negi": negi_np,
        })
    res = run_bass_kernel_spmd(nc, in_maps, core_ids=list(range(N_CORES)))
    rows = np.concatenate([res.results[c]["out"] for c in range(N_CORES)], 0)
    norms = np.linalg.norm(rows.astype(np.float64), axis=-1, keepdims=True)
    rows = rows / (norms + 1e-8)
    sh = np.asarray(selections).shape
    return rows.reshape(sh[0], sh[1], sh[2], NFR * STEP).astype(np.float32)
